# revision 21
# baseline (speedup 1.0000x reference)
"""Trainium2 Bass kernel for nn_DNM_Linear.

Computes, for x[128, 8, 512] (batch, M, IN) and DNM_W[256, 8, 512] (OUT, M, IN):
    z[i, b]   = prod_j sum_k sigmoid(x[i, j, k] * W[b, j, k])
    zn        = z / z.sum(axis=1, keepdims=True)
    out[i, b] = (zn - zn.mean(1, kd)) / zn.std(1, ddof=1, kd)

Sharding: batch dim (128) split across 8 cores (16 rows each). Each core owns
full output rows, so the dim=1 normalize is purely core-local - no collectives.

Per-core pipeline (engines balanced so ScalarE, which must evaluate all
16 * 256 * 8 * 512 sigmoids at 1 elem/lane/cycle, is the only near-saturated
engine):
  - DVE builds diag(x[i,j,ck]) bf16 tiles from an identity via tensor_scalar
    (4x mode).
  - PE computes products x*W via diag-matmuls: stationary = W tile
    [k=128, b_half=128] bf16, moving = 4 batch rows' diags packed [128, 512]
    -> PSUM [b_half, (4i, k)] fp32 (products of bf16-rounded inputs, exact).
  - ScalarE applies Sigmoid on [128, 2048] PSUM reads -> bf16 SBUF. This is
    the bottleneck engine (~121us busy, fully pipelined with zero gaps).
  - DVE tensor_scalar(mult 1.0, accum_out) sums over k (512) at 4x -> S[b, (i,j)].
  - DVE pairwise-mul tree over j=8 -> per-branch product P[b_half, i],
    emitted per wave of 4 batch rows so it overlaps the next wave.
  - PE transpose -> z rows [16, 256]; all-DVE stats epilogue (fused
    copy+row-sum, fused square+accumulate, Newton rsqrt via the bit-trick
    seed - avoids switching the activation table away from the sigmoid set)
    implements the normalize + unbiased standardize.
"""

import numpy as np
import ml_dtypes
from contextlib import ExitStack

BATCH, OUT, M, IN = 128, 256, 8, 512
NCORES = 8
IB = BATCH // NCORES      # 16 batch rows per core
NCK = IN // 128           # 4 k-chunks
NH = OUT // 128           # 2 output halves
NWAVE = IB // 4           # 4 waves of 4 batch rows

_CACHE = {}


def _build():
    """Build + compile the Bass program once. Returns (nc, meta)."""
    import concourse.bass as bass
    import concourse.tile as tile
    from concourse import bacc, mybir
    from concourse.masks import make_identity

    f32 = mybir.dt.float32
    bf16 = mybir.dt.bfloat16
    F = mybir.ActivationFunctionType
    A = mybir.AluOpType

    nc = bacc.Bacc("TRN2", target_bir_lowering=False, debug=False,
                   num_devices=NCORES)

    wt = nc.dram_tensor("wt", [128, M, NCK, NH, 128], bf16,
                        kind="ExternalInput").ap()
    xt = nc.dram_tensor("xt", [128, NWAVE, M, NCK, 4], f32,
                        kind="ExternalInput").ap()
    idb = nc.dram_tensor("idb", [128, 128], bf16, kind="ExternalInput").ap()
    zout = nc.dram_tensor("zout", [IB, OUT], f32, kind="ExternalOutput").ap()

    with tile.TileContext(nc) as tc, ExitStack() as ctx:
        singles = ctx.enter_context(tc.tile_pool(name="singles", bufs=1))
        diagp = ctx.enter_context(tc.tile_pool(name="diagp", bufs=16))
        psump = ctx.enter_context(tc.tile_pool(name="psump", bufs=2,
                                               space="PSUM"))
        zsigp = ctx.enter_context(tc.tile_pool(name="zsigp", bufs=8))

        # DMA order tuned so the first wave's dependencies land first
        # (SP HWDGE queue for the startup-critical pieces; Pool SWDGE
        # issues cost ~1us each so the bulk W goes there, off the
        # critical path).
        xt_s = singles.tile([128, NWAVE, M, NCK, 4], f32, tag="xt", name="xt")
        wt_s = singles.tile([128, M, NCK, NH, 128], bf16, tag="wt", name="wt")
        idb_s = singles.tile([128, 128], bf16, tag="idb", name="idb")
        nc.sync.dma_start(idb_s[:], idb[:])
        nc.sync.dma_start(xt_s[:], xt[:])
        nc.sync.dma_start(wt_s[:, 0, 0], wt[:, 0, 0])
        nc.sync.dma_start(wt_s[:, 1], wt[:, 1])
        nc.gpsimd.dma_start(wt_s[:, 0, 1:4], wt[:, 0, 1:4])
        nc.gpsimd.dma_start(wt_s[:, 2:5], wt[:, 2:5])
        nc.gpsimd.dma_start(wt_s[:, 5:8], wt[:, 5:8])
        idf_s = singles.tile([128, 128], f32, tag="idf", name="idf")
        make_identity(nc, idf_s[:])

        # Per-branch sigmoid-sums: S[h][b_local, i, j]
        S = [singles.tile([128, IB, M], f32, tag=f"S{h}", name=f"S{h}") for h in range(NH)]
        junk_b = singles.tile([128, 512], bf16, tag="junkb", name="junkb")
        # j-products, built per wave: P[h][b_local, i] (i padded to 32)
        P = [singles.tile([128, 32], f32, tag=f"P{h}", name=f"P{h}") for h in range(NH)]
        for h in range(NH):
            nc.vector.memset(P[h][:], 0.0)

        # ---------------- main loop ----------------
        for wave in range(NWAVE):
            for j in range(M):
                d4 = []
                for ck in range(NCK):
                    d = diagp.tile([128, 4, 128], bf16, tag="diag", name="diag")
                    for il in range(4):
                        nc.vector.tensor_scalar(
                            d[:, il, :], idb_s[:],
                            xt_s[:, wave, j, ck, il : il + 1], None, A.mult)
                    d4.append(d)
                for h in range(NH):
                    pt = psump.tile([128, NCK, 4, 128], f32, tag="pt", name="pt")
                    for ck in range(NCK):
                        nc.tensor.matmul(
                            pt[:, ck, :, :],
                            wt_s[:, j, ck, h, :],
                            d4[ck][:],
                            start=True, stop=True)
                    zt = zsigp.tile([128, NCK, 4, 128], bf16, tag="zt", name="zt")
                    nc.scalar.activation(zt[:], pt[:], F.Sigmoid)
                    for il in range(4):
                        i = wave * 4 + il
                        nc.vector.tensor_scalar(
                            junk_b[:], zt[:, :, il, :], 1.0, None, A.mult,
                            A.add, accum_out=S[h][:, i, j : j + 1])
            # per-wave j-product tree (overlaps next wave's compute)
            w4 = wave * 4
            for h in range(NH):
                r1 = singles.tile([128, 4, 4], f32, tag=f"r1_{h}", name=f"r1_{h}")
                nc.vector.tensor_mul(r1[:], S[h][:, w4:w4 + 4, 0:4],
                                     S[h][:, w4:w4 + 4, 4:8])
                r2 = singles.tile([128, 4, 2], f32, tag=f"r2_{h}", name=f"r2_{h}")
                nc.vector.tensor_mul(r2[:], r1[:, :, 0:2], r1[:, :, 2:4])
                nc.vector.tensor_mul(P[h][:, w4:w4 + 4], r2[:, :, 0:1],
                                     r2[:, :, 1:2])

        # ---------------- epilogue ----------------
        # transpose -> z rows [16, 256] in PSUM
        zT = psump.tile([32, OUT], f32, tag="pt", name="zT")
        for h in range(NH):
            nc.tensor.transpose(zT[0:32, h * 128:(h + 1) * 128],
                                P[h][:], idf_s[:])
        # copy PSUM->SBUF fused with row-sum accumulate (all-DVE epilogue)
        zS = singles.tile([IB, OUT], f32, tag="zS", name="zS")
        tot = singles.tile([IB, 1], f32, tag="tot", name="tot")
        nc.vector.tensor_scalar(zS[:], zT[0:IB, :], 1.0, None, A.mult,
                                A.add, accum_out=tot[:])
        rT = singles.tile([IB, 1], f32, tag="rT", name="rT")
        nc.vector.reciprocal(rT[:], tot[:])
        # ssz = sum(zn * z) = total * sum(zn^2);  out tensor itself is junk
        junk32 = singles.tile([IB, OUT], f32, tag="junk32", name="junk32")
        ssz = singles.tile([IB, 1], f32, tag="ssz", name="ssz")
        nc.vector.scalar_tensor_tensor(junk32[:], zS[:], rT[:], zS[:],
                                       A.mult, A.mult, accum_out=ssz[:])
        # var = (ssz/total - 1/256) / 255  (unbiased; mean is exactly 1/256)
        ssn = singles.tile([IB, 1], f32, tag="ssn", name="ssn")
        nc.vector.tensor_mul(ssn[:], ssz[:], rT[:])
        var = singles.tile([IB, 1], f32, tag="var", name="var")
        nc.vector.tensor_scalar(var[:], ssn[:], 1.0 / OUT, 1.0 / (OUT - 1),
                                A.subtract, A.mult)
        # rstd = rsqrt(var): DVE-only Newton (no activation-table switch)
        u32 = mybir.dt.uint32
        Cs = singles.tile([IB, 1], u32, tag="Cs", name="Cs")
        nc.vector.memset(Cs[:], 0x5F3759DF)
        u1 = singles.tile([IB, 1], u32, tag="u1", name="u1")
        nc.vector.tensor_scalar(u1[:], var[:].bitcast(u32), 1, None,
                                A.logical_shift_right)
        y0u = singles.tile([IB, 1], u32, tag="y0u", name="y0u")
        nc.vector.tensor_sub(y0u[:], Cs[:], u1[:])
        ycur = singles.tile([IB, 1], f32, tag="ycur", name="ycur")
        nc.vector.tensor_copy(ycur[:], y0u[:].bitcast(f32))
        for it in range(2):
            tn = singles.tile([IB, 1], f32, tag=f"tn{it}", name=f"tn{it}")
            nc.vector.tensor_mul(tn[:], ycur[:], ycur[:])
            nc.vector.tensor_mul(tn[:], tn[:], var[:])
            nc.vector.tensor_scalar(tn[:], tn[:], -0.5, 1.5, A.mult, A.add)
            yn = singles.tile([IB, 1], f32, tag=f"yn{it}", name=f"yn{it}")
            nc.vector.tensor_mul(yn[:], ycur[:], tn[:])
            ycur = yn
        rstd = ycur
        # out = z * (rT*rstd) + (-rstd/256)
        alpha = singles.tile([IB, 1], f32, tag="alpha", name="alpha")
        nc.vector.tensor_mul(alpha[:], rT[:], rstd[:])
        beta = singles.tile([IB, 1], f32, tag="beta", name="beta")
        nc.vector.tensor_scalar(beta[:], rstd[:], -1.0 / OUT, None, A.mult)
        outS = singles.tile([IB, OUT], f32, tag="outS", name="outS")
        nc.vector.tensor_scalar(outS[:], zS[:], alpha[:], beta[:],
                                A.mult, A.add)
        nc.sync.dma_start(zout[:], outS[:])

    nc.compile()
    return nc


def get_nc():
    if "nc" not in _CACHE:
        _CACHE["nc"] = _build()
    return _CACHE["nc"]


def prep_inputs(x: np.ndarray, DNM_W: np.ndarray):
    """Host-side packing into the layouts the kernel wants."""
    bf = ml_dtypes.bfloat16
    # wt[p, j, ck, h, m] = W[h*128 + m, j, ck*128 + p]
    w = DNM_W.reshape(NH, 128, M, NCK, 128)          # (h, m, j, ck, p)
    wt = np.ascontiguousarray(w.transpose(4, 2, 3, 0, 1)).astype(bf)
    in_maps = []
    for c in range(NCORES):
        xc = x[c * IB:(c + 1) * IB]                   # (16, 8, 512)
        xr = xc.reshape(NWAVE, 4, M, NCK, 128)        # (w, il, j, ck, p)
        xts = np.ascontiguousarray(
            xr.transpose(4, 0, 2, 3, 1)).astype(np.float32)
        in_maps.append({"wt": wt, "xt": xts,
                        "idb": np.eye(128, dtype=bf)})
    return in_maps


def kernel(x: np.ndarray, DNM_W: np.ndarray, **run_kwargs) -> np.ndarray:
    from concourse import bass_utils

    x = np.asarray(x, dtype=np.float32)
    DNM_W = np.asarray(DNM_W, dtype=np.float32)
    nc = get_nc()
    in_maps = prep_inputs(x, DNM_W)
    res = bass_utils.run_bass_kernel_spmd(
        nc, in_maps, core_ids=list(range(NCORES)), **run_kwargs)
    out = np.concatenate([np.asarray(r["zout"]) for r in res.results], axis=0)
    if run_kwargs:
        _CACHE["last_results"] = res
    return out


# revision 29
# speedup vs baseline: 1.0013x; 1.0013x over previous
"""Trainium2 Bass kernel for nn_DNM_Linear.

Computes, for x[128, 8, 512] (batch, M, IN) and DNM_W[256, 8, 512] (OUT, M, IN):
    z[i, b]   = prod_j sum_k sigmoid(x[i, j, k] * W[b, j, k])
    zn        = z / z.sum(axis=1, keepdims=True)
    out[i, b] = (zn - zn.mean(1, kd)) / zn.std(1, ddof=1, kd)

Sharding: batch dim (128) split across 8 cores (16 rows each). Each core owns
full output rows, so the dim=1 normalize is purely core-local - no collectives.

Per-core pipeline (engines balanced so ScalarE, which must evaluate all
16 * 256 * 8 * 512 sigmoids at 1 elem/lane/cycle, is the only near-saturated
engine):
  - DVE builds diag(x[i,j,ck]) bf16 tiles from an identity via tensor_scalar
    (4x mode).
  - PE computes products x*W via diag-matmuls: stationary = W tile
    [k=128, b_half=128] bf16, moving = 4 batch rows' diags packed [128, 512]
    -> PSUM [b_half, (4i, k)] fp32 (products of bf16-rounded inputs, exact).
  - ScalarE applies Sigmoid on [128, 2048] PSUM reads -> bf16 SBUF. This is
    the bottleneck engine (~121us busy, fully pipelined with zero gaps).
  - DVE tensor_scalar(mult 1.0, accum_out) sums over k (512) at 4x -> S[b, (i,j)].
  - DVE pairwise-mul tree over j=8 -> per-branch product P[b_half, i],
    emitted per wave of 4 batch rows so it overlaps the next wave.
  - PE transpose -> z rows [16, 256]; all-DVE stats epilogue (fused
    copy+row-sum, fused square+accumulate, Newton rsqrt via the bit-trick
    seed - avoids switching the activation table away from the sigmoid set)
    implements the normalize + unbiased standardize.
"""

import numpy as np
import ml_dtypes
from contextlib import ExitStack

BATCH, OUT, M, IN = 128, 256, 8, 512
NCORES = 8
IB = BATCH // NCORES      # 16 batch rows per core
NCK = IN // 128           # 4 k-chunks
NH = OUT // 128           # 2 output halves
NWAVE = IB // 4           # 4 waves of 4 batch rows

_CACHE = {}


def _build():
    """Build + compile the Bass program once. Returns (nc, meta)."""
    import concourse.bass as bass
    import concourse.tile as tile
    from concourse import bacc, mybir
    from concourse.masks import make_identity
    from concourse.tile import add_dep_helper

    f32 = mybir.dt.float32
    bf16 = mybir.dt.bfloat16
    F = mybir.ActivationFunctionType
    A = mybir.AluOpType

    nc = bacc.Bacc("TRN2", target_bir_lowering=False, debug=False,
                   num_devices=NCORES)

    wt = nc.dram_tensor("wt", [128, M, NCK, NH, 128], bf16,
                        kind="ExternalInput").ap()
    xt = nc.dram_tensor("xt", [128, NWAVE, M, NCK, 4], f32,
                        kind="ExternalInput").ap()
    idb = nc.dram_tensor("idb", [128, 128], bf16, kind="ExternalInput").ap()
    zout = nc.dram_tensor("zout", [IB, OUT], f32, kind="ExternalOutput").ap()

    with tile.TileContext(nc) as tc, ExitStack() as ctx:
        singles = ctx.enter_context(tc.tile_pool(name="singles", bufs=1))
        diagp = ctx.enter_context(tc.tile_pool(name="diagp", bufs=16))
        psump = ctx.enter_context(tc.tile_pool(name="psump", bufs=2,
                                               space="PSUM"))
        zsigp = ctx.enter_context(tc.tile_pool(name="zsigp", bufs=8))

        # DMA order tuned so the first wave's dependencies land first
        # (SP HWDGE queue for the startup-critical pieces; Pool SWDGE
        # issues cost ~1us each so the bulk W goes there, off the
        # critical path).
        xt_s = singles.tile([128, NWAVE, M, NCK, 4], f32, tag="xt", name="xt")
        wt_s = singles.tile([128, M, NCK, NH, 128], bf16, tag="wt", name="wt")
        idb_s = singles.tile([128, 128], bf16, tag="idb", name="idb")
        nc.gpsimd.dma_start(idb_s[:], idb[:])
        nc.sync.dma_start(xt_s[:], xt[:])
        nc.sync.dma_start(wt_s[:, 0, 0], wt[:, 0, 0])
        nc.sync.dma_start(wt_s[:, 1], wt[:, 1])
        nc.gpsimd.dma_start(wt_s[:, 0, 1:4], wt[:, 0, 1:4])
        nc.gpsimd.dma_start(wt_s[:, 2:5], wt[:, 2:5])
        nc.gpsimd.dma_start(wt_s[:, 5:8], wt[:, 5:8])
        idf_s = singles.tile([128, 128], f32, tag="idf", name="idf")
        make_identity(nc, idf_s[:])

        # Per-branch sigmoid-sums: S[h][b_local, i, j]
        S = [singles.tile([128, IB, M], f32, tag=f"S{h}", name=f"S{h}") for h in range(NH)]
        junk_b = singles.tile([128, 512], bf16, tag="junkb", name="junkb")

        # j-products, built per wave: P[h][b_local, i] (i padded to 32)
        P = [singles.tile([128, 32], f32, tag=f"P{h}", name=f"P{h}") for h in range(NH)]
        for h in range(NH):
            nc.vector.memset(P[h][:], 0.0)

        # ---------------- main loop ----------------
        for wave in range(NWAVE):
            for j in range(M):
                d4 = []
                for ck in range(NCK):
                    d = diagp.tile([128, 4, 128], bf16, tag="diag", name="diag")
                    for il in range(4):
                        nc.vector.tensor_scalar(
                            d[:, il, :], idb_s[:],
                            xt_s[:, wave, j, ck, il : il + 1], None, A.mult)
                    d4.append(d)
                for h in range(NH):
                    pt = psump.tile([128, NCK, 4, 128], f32, tag="pt", name="pt")
                    for ck in range(NCK):
                        mm = nc.tensor.matmul(
                            pt[:, ck, :, :],
                            wt_s[:, j, ck, h, :],
                            d4[ck][:],
                            start=True, stop=True)
                        if wave == 0 and j == 0:
                            # at cold PE clocks, don't let h=1 matmuls slot in
                            # ahead of the first group's completion
                            if h == 0 and ck == NCK - 1:
                                first_last_mm = mm
                            if h == 1:
                                add_dep_helper(mm.ins, first_last_mm.ins,
                                               sync=False,
                                               reason="warmup order h0<h1")
                    zt = zsigp.tile([128, NCK, 4, 128], bf16, tag="zt", name="zt")
                    nc.scalar.activation(zt[:], pt[:], F.Sigmoid)
                    for il in range(4):
                        i = wave * 4 + il
                        nc.vector.tensor_scalar(
                            junk_b[:], zt[:, :, il, :], 1.0, None, A.mult,
                            A.add, accum_out=S[h][:, i, j : j + 1])
            # per-wave j-product tree (overlaps next wave's compute)
            w4 = wave * 4
            for h in range(NH):
                r1 = singles.tile([128, 4, 4], f32, tag=f"r1_{h}", name=f"r1_{h}")
                nc.vector.tensor_mul(r1[:], S[h][:, w4:w4 + 4, 0:4],
                                     S[h][:, w4:w4 + 4, 4:8])
                r2 = singles.tile([128, 4, 2], f32, tag=f"r2_{h}", name=f"r2_{h}")
                nc.vector.tensor_mul(r2[:], r1[:, :, 0:2], r1[:, :, 2:4])
                nc.vector.tensor_mul(P[h][:, w4:w4 + 4], r2[:, :, 0:1],
                                     r2[:, :, 1:2])

        # ---------------- epilogue ----------------
        # transpose -> z rows [16, 256] in PSUM
        zT = psump.tile([32, OUT], f32, tag="pt", name="zT")
        for h in range(NH):
            nc.tensor.transpose(zT[0:32, h * 128:(h + 1) * 128],
                                P[h][:], idf_s[:])
        # copy PSUM->SBUF fused with row-sum accumulate (all-DVE epilogue)
        zS = singles.tile([IB, OUT], f32, tag="zS", name="zS")
        tot = singles.tile([IB, 1], f32, tag="tot", name="tot")
        nc.vector.tensor_scalar(zS[:], zT[0:IB, :], 1.0, None, A.mult,
                                A.add, accum_out=tot[:])
        rT = singles.tile([IB, 1], f32, tag="rT", name="rT")
        nc.vector.reciprocal(rT[:], tot[:])
        # ssz = sum(zn * z) = total * sum(zn^2);  out tensor itself is junk
        junk32 = singles.tile([IB, OUT], f32, tag="junk32", name="junk32")
        ssz = singles.tile([IB, 1], f32, tag="ssz", name="ssz")
        nc.vector.scalar_tensor_tensor(junk32[:], zS[:], rT[:], zS[:],
                                       A.mult, A.mult, accum_out=ssz[:])
        # q = ssz/total - 1/256 = 255 * var  (unbiased var; mean = 1/256
        # exactly). The 1/255 is folded into alpha/beta as sqrt(255) below.
        var = singles.tile([IB, 1], f32, tag="var", name="var")
        nc.vector.tensor_scalar(var[:], ssz[:], rT[:], 1.0 / OUT,
                                A.mult, A.subtract)
        # rstd = rsqrt(var): DVE-only Newton (no activation-table switch)
        u32 = mybir.dt.uint32
        Cs = singles.tile([IB, 1], u32, tag="Cs", name="Cs")
        nc.vector.memset(Cs[:], 0x5F3759DF)
        u1 = singles.tile([IB, 1], u32, tag="u1", name="u1")
        nc.vector.tensor_scalar(u1[:], var[:].bitcast(u32), 1, None,
                                A.logical_shift_right)
        y0u = singles.tile([IB, 1], u32, tag="y0u", name="y0u")
        nc.vector.tensor_sub(y0u[:], Cs[:], u1[:])
        ycur = singles.tile([IB, 1], f32, tag="ycur", name="ycur")
        nc.vector.tensor_copy(ycur[:], y0u[:].bitcast(f32))
        for it in range(2):
            tn = singles.tile([IB, 1], f32, tag=f"tn{it}", name=f"tn{it}")
            nc.vector.tensor_mul(tn[:], ycur[:], ycur[:])
            nc.vector.tensor_mul(tn[:], tn[:], var[:])
            nc.vector.tensor_scalar(tn[:], tn[:], -0.5, 1.5, A.mult, A.add)
            yn = singles.tile([IB, 1], f32, tag=f"yn{it}", name=f"yn{it}")
            nc.vector.tensor_mul(yn[:], ycur[:], tn[:])
            ycur = yn
        rstd = ycur
        # out = z * (rT*rstd) + (-rstd/256), rstd = sqrt(255) * rsqrt(q)
        SQ = float(np.sqrt(OUT - 1.0))
        alpha = singles.tile([IB, 1], f32, tag="alpha", name="alpha")
        nc.vector.scalar_tensor_tensor(alpha[:], rT[:], SQ, rstd[:],
                                       A.mult, A.mult)
        beta = singles.tile([IB, 1], f32, tag="beta", name="beta")
        nc.vector.tensor_scalar(beta[:], rstd[:], -SQ / OUT, None, A.mult)
        outS = singles.tile([IB, OUT], f32, tag="outS", name="outS")
        nc.vector.tensor_scalar(outS[:], zS[:], alpha[:], beta[:],
                                A.mult, A.add)
        nc.sync.dma_start(zout[:], outS[:])

    nc.compile()
    return nc


def get_nc():
    if "nc" not in _CACHE:
        _CACHE["nc"] = _build()
    return _CACHE["nc"]


def prep_inputs(x: np.ndarray, DNM_W: np.ndarray):
    """Host-side packing into the layouts the kernel wants."""
    bf = ml_dtypes.bfloat16
    # wt[p, j, ck, h, m] = W[h*128 + m, j, ck*128 + p]
    w = DNM_W.reshape(NH, 128, M, NCK, 128)          # (h, m, j, ck, p)
    wt = np.ascontiguousarray(w.transpose(4, 2, 3, 0, 1)).astype(bf)
    in_maps = []
    for c in range(NCORES):
        xc = x[c * IB:(c + 1) * IB]                   # (16, 8, 512)
        xr = xc.reshape(NWAVE, 4, M, NCK, 128)        # (w, il, j, ck, p)
        xts = np.ascontiguousarray(
            xr.transpose(4, 0, 2, 3, 1)).astype(np.float32)
        in_maps.append({"wt": wt, "xt": xts,
                        "idb": np.eye(128, dtype=bf)})
    return in_maps


def kernel(x: np.ndarray, DNM_W: np.ndarray, **run_kwargs) -> np.ndarray:
    from concourse import bass_utils

    x = np.asarray(x, dtype=np.float32)
    DNM_W = np.asarray(DNM_W, dtype=np.float32)
    nc = get_nc()
    in_maps = prep_inputs(x, DNM_W)
    res = bass_utils.run_bass_kernel_spmd(
        nc, in_maps, core_ids=list(range(NCORES)), **run_kwargs)
    out = np.concatenate([np.asarray(r["zout"]) for r in res.results], axis=0)
    if run_kwargs:
        _CACHE["last_results"] = res
    return out
